# revision 68
# baseline (speedup 1.0000x reference)
# Trainium2 Bass kernel: single-head causal attention (k.q^T scores, no scale)
# B=16, T=4096, D=64. Data-parallel over batch: 2 batches per NeuronCore x 8.
#
# v2 structure (per batch):
#   - q,k projections -> qTd/kTd [128, T] f32r (dup on partition halves)
#   - v projection (bf16) -> vex [128, NSB, 65] bf16 (col 64 = ones for the
#     softmax denominator)
#   - per t-tile (512 cols): stream s-block pairs; scores via 2 row-half
#     matmuls into PSUM st [128,1024]; exp via ACT (or Schraudolph bit-trick
#     on DVE + GPSIMD convert for some chunks, to spread the softmax work
#     over three engines); causal mask on the diagonal 128-blocks only;
#     PV as bf16 matmuls with P as weights (N=65) accumulating into 4
#     interleaved PSUM groups [128, 65] -> output lands in [t, h] layout, so
#     the drain is just reciprocal + scale + DMA (no transpose).
import numpy as np

B, T, D = 16, 4096, 64
NCORES = 8
BPC = B // NCORES      # batches per core
TT = 512               # t-tile width
NTT = T // TT          # 8 t tiles
SB = 128               # s block
NSB = T // SB          # 32 s blocks

# Schraudolph fast-exp constants: exp(x) ~= bits_as_f32(int32(x*EA + EB))
EA = 12102203.161561485            # 2^23 / ln 2
EB = float(127 * 2 ** 23 - 490000)
OFF_PAT = (False, True, False, True, False, True, False)  # offload 3 of 7

_cache = {}


SPLIT_WAITS = True     # set False for CoreSim value-testing
WARMUP = True
FAST_COPY = True


def _build():
    from contextlib import ExitStack
    import concourse.bass as bass
    import concourse.mybir as mybir
    import concourse.tile as tile

    f32 = mybir.dt.float32
    f32r = mybir.dt.float32r
    bf16 = mybir.dt.bfloat16
    i32 = mybir.dt.int32
    EXP = mybir.ActivationFunctionType.Exp
    MULT = mybir.AluOpType.mult
    ADD = mybir.AluOpType.add

    nc = bass.Bass("TRN2", target_bir_lowering=False, debug=False,
                   enable_asserts=False)

    xT_d = nc.dram_tensor("xt", [BPC, D, T], f32r, kind="ExternalInput").ap()
    xb_d = nc.dram_tensor("xbf", [BPC, D, T], bf16, kind="ExternalInput").ap()
    wq_d = nc.dram_tensor("wqt2", [D, 128], f32r, kind="ExternalInput").ap()
    wk_d = nc.dram_tensor("wkt2", [D, 128], f32r, kind="ExternalInput").ap()
    wv_d = nc.dram_tensor("wvb", [D, D], bf16, kind="ExternalInput").ap()
    mk_d = nc.dram_tensor("mkb", [128, 128], bf16, kind="ExternalInput").ap()
    on_d = nc.dram_tensor("onesb", [128, NSB], bf16, kind="ExternalInput").ap()
    # p-major output layout: host transposes (b, t, p, tch, h) -> (b, t*512
    # + tch*128 + p, h); lets each tile's 4 normalized blocks ship as ONE DMA
    out_d = nc.dram_tensor("out", [BPC, NTT, SB, 4, D], f32,
                           kind="ExternalOutput").ap()

    with ExitStack() as ctx:
        tc = ctx.enter_context(tile.TileContext(nc))
        consts = ctx.enter_context(tc.tile_pool(name="consts", bufs=1))
        bigp = ctx.enter_context(tc.tile_pool(name="big", bufs=2))
        ptp = ctx.enter_context(tc.tile_pool(name="pt", bufs=8))
        tmpp = ctx.enter_context(tc.tile_pool(name="tmp", bufs=4))
        stg = ctx.enter_context(tc.tile_pool(name="stg", bufs=4))
        # PSUM: st [128,1024]x3 = 6 banks, acc [128,260]x2 = 2 banks
        pst = ctx.enter_context(tc.tile_pool(name="pst", bufs=3, space="PSUM"))
        pacc = ctx.enter_context(tc.tile_pool(name="pacc", bufs=2, space="PSUM"))

        wq_sb = consts.tile([D, 128], f32r, tag="wq")
        wk_sb = consts.tile([D, 128], f32r, tag="wk")
        wv_sb = consts.tile([D, D], bf16, tag="wv")
        mk_sb = consts.tile([128, 128], bf16, tag="mk")

        # PE warmup during the input-DMA wait: ~3.5us of dummy matmuls on
        # zeroed scratch pulls the HAM clock gate to 8/8 before real work
        if WARMUP:
            # plain f32 dummies (fp32r operands would need f32r-rounded
            # producers, which memset can't provide)
            dumw = consts.tile([D, 512], f32, tag="dumw")
            nc.vector.memset(dumw[:], 0.0)
            dum_ps = pacc.tile([128, 260], f32, tag="acc", name="warm")
            for _ in range(4):
                nc.tensor.matmul(dum_ps[:, 0:260], dumw[:, 0:128],
                                 dumw[:, 0:260])

        off_ctr = [0]

        # ---- load x^T for both batches (f32 for qk, bf16 for v), chunked
        # so projection tile i only waits on its own slice; ordered by first
        # use and issued round-robin on the SP and Pool (SWDGE) queues so
        # neither sequencer serializes the startup
        xt_sb, xb_sb, qTd, kTd, vex = {}, {}, {}, {}, {}
        for b in range(BPC):
            xt_sb[b] = bigp.tile([D, T], f32r, tag="xt", name=f"xt{b}")
            xb_sb[b] = bigp.tile([D, T], bf16, tag="xb", name=f"xb{b}")
            qTd[b] = bigp.tile([128, T], f32r, tag="qtd", name=f"qtd{b}")
            kTd[b] = bigp.tile([128, T], f32r, tag="ktd", name=f"ktd{b}")
            vex[b] = bigp.tile([128, NSB, 65], bf16, tag="vex", name=f"vex{b}")
        loads = [
            # startup-critical first: tiny consts + the first x chunks
            (wq_sb[:], wq_d[:]),            # sync
            (xb_sb[0][:, 0:2048], xb_d[0][:, 0:2048]),   # gpsimd
            (xt_sb[0][:, 0:1024], xT_d[0][:, 0:1024]),   # sync
            (wv_sb[:], wv_d[:]),            # gpsimd
            (wk_sb[:], wk_d[:]),            # sync
            (mk_sb[:], mk_d[:]),            # gpsimd
            (xt_sb[1][:, 0:1024], xT_d[1][:, 0:1024]),   # sync
            (xb_sb[1][:, 0:2048], xb_d[1][:, 0:2048]),   # gpsimd
            (vex[0][:, :, 64], on_d[:]),
            (vex[1][:, :, 64], on_d[:]),
        ]
        for i in range(1, 4):
            for b in range(BPC):
                sl = slice(i * 1024, (i + 1) * 1024)
                loads.append((xt_sb[b][:, sl], xT_d[b][:, sl]))
                if i == 1:
                    loads.append((xb_sb[b][:, 2048:T], xb_d[b][:, 2048:T]))
        for n, (dst, src) in enumerate(loads):
            (nc.sync if n % 2 == 0 else nc.gpsimd).dma_start(dst, src)

        def chunk_front(b, t, c):
            """Scores + exp + mask for chunk (b,t,c). Returns the pt source
            for the (deferred) PV stage: an AP factory col -> [128,128]."""
            st = pst.tile([128, 1024], f32, tag="st")
            diag2 = (c == 2 * t + 1)   # j = 2,3 chunk
            # score matmuls (row-tiled halves); trim fully-masked cols
            # but keep N >= 256 (fp32r full-rate threshold)
            for p in range(2):
                sblk = 2 * c + p
                j = sblk - 4 * t
                # trim only when the exp is trimmed too (j>=2), so the
                # activation never reads columns this tile didn't write
                lo = 256 if j >= 2 else 0
                half = slice(64 * p, 64 * (p + 1))
                nc.tensor.matmul(
                    st[:, 512 * p + lo: 512 * (p + 1)],
                    qTd[b][half, sblk * SB:(sblk + 1) * SB],
                    kTd[b][half, t * TT + lo:(t + 1) * TT])
            # exp
            if not diag2 and c < 2 * t and OFF_PAT[off_ctr[0] % len(OFF_PAT)]:
                off_ctr[0] += 1
                # Schraudolph fast-exp on DVE: int32(x*EA+EB) are the f32
                # bits of exp(x); PV reads the top half of each word as bf16
                tmp = tmpp.tile([128, 1024, 2], bf16, tag="tmp")
                nc.vector.tensor_scalar(
                    tmp[:, :, :].bitcast(i32)[:, :, 0], st[:], EA, EB,
                    MULT, ADD)
                return lambda lo: tmp[:, lo:lo + 128, 1]
            pt = ptp.tile([128, 1024], bf16, tag="pt")
            if diag2:
                # only cols >= 128*j are live: two trimmed ACT ops
                nc.scalar.activation(pt[:, 256:512], st[:, 256:512], EXP)
                nc.scalar.activation(pt[:, 896:1024], st[:, 896:1024], EXP)
            else:
                if c < 2 * t:
                    off_ctr[0] += 1
                nc.scalar.activation(pt[:], st[:], EXP)
            # causal mask on diagonal 128-blocks (split across DVE and the
            # otherwise-idle GPSIMD so neither serializes the boundary)
            for p in range(2):
                j = 2 * c + p - 4 * t
                if 0 <= j <= 3:
                    lo = 512 * p + 128 * j
                    if b == 0:
                        nc.vector.tensor_mul(pt[:, lo:lo + 128],
                                             pt[:, lo:lo + 128], mk_sb[:])
                    else:
                        nc.gpsimd.tensor_tensor(pt[:, lo:lo + 128],
                                                pt[:, lo:lo + 128], mk_sb[:],
                                                mybir.AluOpType.mult)
            return lambda lo: pt[:, lo:lo + 128]

        def chunk_pv(b, t, c, acc, src, ctr, total):
            # PV: P as weights, V streaming (bf16, N=65). The four tch
            # sub-regions share one PSUM bank = one zero region: only the
            # bank's first matmul may carry start (it clears has_written for
            # the WHOLE 2KB region) and only the last may carry stop; in
            # between, per-element has_written gives each sub-region
            # overwrite-then-accumulate.
            for p in range(2):
                sblk = 2 * c + p
                j = sblk - 4 * t
                for tch in range(4):
                    if j > tch:
                        continue   # fully masked block
                    nc.tensor.matmul(
                        acc[:, 65 * tch: 65 * tch + 65],
                        src(512 * p + 128 * tch),
                        vex[b][:, sblk, :],
                        start=(ctr[0] == 0),
                        stop=(ctr[0] == total - 1))
                    ctr[0] += 1

        def n_pv(t):
            total = 0
            for c in range(2 * (t + 1)):
                for p in range(2):
                    j = 2 * c + p - 4 * t
                    total += 4 - max(j, 0)
            return total

        def proj(b, t, fast=False):
            ps = pst.tile([128, 1024], f32, tag="st")
            sl = slice(t * TT, (t + 1) * TT)
            nc.tensor.matmul(ps[:, 0:512], wq_sb[:], xt_sb[b][:, sl])
            nc.tensor.matmul(ps[:, 512:1024], wk_sb[:], xt_sb[b][:, sl])
            if fast and FAST_COPY:
                # startup only: ACT is idle, so split the two drains across
                # ACT and DVE instead of serializing on DVE
                nc.scalar.copy(qTd[b][:, sl], ps[:, 0:512])
            else:
                nc.vector.tensor_copy(qTd[b][:, sl], ps[:, 0:512])
            nc.vector.tensor_copy(kTd[b][:, sl], ps[:, 512:1024])

        def vproj(b, g):         # v proj: 8 token-blocks of 128 per group
            psv = pst.tile([128, 8, 64], f32, tag="st")
            for k in range(8):
                tb = 8 * g + k
                nc.tensor.matmul(psv[:, k, :],
                                 xb_sb[b][:, tb * SB:(tb + 1) * SB],
                                 wv_sb[:])
            nc.vector.tensor_copy(
                vex[b][:, 8 * g:8 * (g + 1), 0:64], psv[:, :, :])

        # ---- fused projection + attention, the two batches interleaved as
        # independent streams, PV deferred one chunk behind exp so the PE
        # stream never head-of-line blocks on a pending exp, projection
        # pipelined one t-tile ahead so each tile's kTd is ready early.
        # Tiles processed [1..7, 0]: the tiny tile 0 lands at the kernel
        # tail, shrinking the non-overlappable end of the pipeline.
        TILE_ORDER = [1, 2, 3, 4, 5, 6, 7, 0]
        pending_drain = [None]
        for b in range(BPC):
            proj(b, 0, fast=True)
            proj(b, 1, fast=True)
        for i, t in enumerate(TILE_ORDER):
            nxt = TILE_ORDER[i + 1] if i + 1 < NTT else None
            if nxt is not None and nxt >= 2:
                for b in range(BPC):
                    proj(b, nxt)

            accs = {}
            for b in range(BPC):
                if t == 0:
                    # last tile: take its acc bank from the st pool so it
                    # doesn't wait on tile 7's drain (pacc rotation); only
                    # cols 0:260 of bank 0 are used
                    accs[b] = pst.tile([128, 1024], f32, tag="st",
                                       name=f"acc{b}_{t}")
                else:
                    accs[b] = pacc.tile([128, 260], f32, tag="acc",
                                        name=f"acc{b}_{t}")
            # reversed chunk order: the diagonal chunks (short ACT ops +
            # masks) run at the tile head, overlapped with the previous
            # tile's full-ACT tail, instead of bunching at the boundary.
            # (Forward order for the first section: no previous tail to
            # smooth, and early chunks only need already-done projections.)
            pend, ctr, total = {}, {}, n_pv(t)
            order = list(range(2 * (t + 1)))
            if i > 0:
                order.reverse()
            for k, c in enumerate(order):
                if k == 1 and i == 0:
                    for b in range(BPC):
                        vproj(b, 0)   # blocks 0-7, needed by first PVs
                for b in range(BPC):
                    if k == 0:
                        ctr[b] = [0]
                    src = chunk_front(b, t, c)
                    if k > 0:
                        chunk_pv(b, t, order[k - 1], accs[b], pend[b],
                                 ctr[b], total)
                    pend[b] = src
            for b in range(BPC):
                chunk_pv(b, t, order[-1], accs[b], pend[b], ctr[b], total)
            if i <= 2:
                # v proj group i+1 at the section tail: ready well before
                # the NEXT section's (reversed-order) high s-block PVs
                for b in range(BPC):
                    vproj(b, i + 1)

            # ---- drain: normalize + one batched store per (b, t)
            def drain(accs=accs, t=t):
                for b in range(BPC):
                    on4 = stg.tile([128, 4, 64], f32, tag="on",
                                   name=f"on4_{b}_{t}")
                    for tch in range(4):
                        rcp = stg.tile([128, 1], f32, tag="rcp",
                                       name=f"rcp_{b}_{t}_{tch}")
                        nc.vector.reciprocal(
                            rcp[:], accs[b][:, 65 * tch + 64: 65 * tch + 65])
                        nc.vector.tensor_scalar_mul(
                            on4[:, tch, :],
                            accs[b][:, 65 * tch: 65 * tch + 64], rcp[:])
                    nc.sync.dma_start(out_d[b, t], on4[:])

            drain()

    if SPLIT_WAITS:
        _split_matmul_waits(nc)
    return nc


def _split_matmul_waits(nc):
    """fp32/fp32r matmuls lower via an LDWEIGHTS struct with a single ISA
    wait slot; walrus refuses Matmult instructions carrying >1 sync wait.
    Move every multi-wait Matmult's waits onto a PE NoOp inserted right
    before it (engines execute their stream in order, so this is
    equivalent)."""
    import bass_rust
    import concourse.mybir as mybir
    moved = 0
    for fn in nc.m.functions:
        for bb in fn.blocks:
            il = bb.instructions
            k = 0
            while k < len(il):
                inst = il[k]
                if inst.opcode != "NoOp":
                    si = inst.sync_info
                    if si is not None and si.on_wait and len(si.on_wait) > 1:
                        waits = list(si.on_wait)
                        ups = list(si.on_update) if si.on_update else []
                        # every TPB instruction has a single ISA wait slot:
                        # one NoOp per wait, in order, before the matmul
                        for wi, w in enumerate(waits):
                            nop = mybir.InstNoOp(name=f"{inst.name}-ws{wi}",
                                                 ins=[], outs=[])
                            nop.engine = inst.engine
                            nop.sync_info = bass_rust.SyncInfo(
                                on_wait=[w], on_update=[])
                            il.insert(k, nop)
                            k += 1
                        inst.sync_info = bass_rust.SyncInfo(
                            on_wait=[], on_update=ups)
                        moved += 1
                k += 1
    return moved


def _get_nc():
    if "nc" not in _cache:
        _cache["nc"] = _build()
    return _cache["nc"]


def kernel(x, Wk, Wq, Wv):
    import ml_dtypes
    from concourse.bass_utils import run_bass_kernel_spmd

    x = np.ascontiguousarray(np.asarray(x, dtype=np.float32))
    Wk = np.asarray(Wk, dtype=np.float32)
    Wq = np.asarray(Wq, dtype=np.float32)
    Wv = np.asarray(Wv, dtype=np.float32)

    xT = np.ascontiguousarray(x.transpose(0, 2, 1))          # [B, D, T]
    xbf = np.ascontiguousarray(xT.astype(ml_dtypes.bfloat16))
    wq2 = np.ascontiguousarray(np.concatenate([Wq.T, Wq.T], axis=1))  # [64,128]
    wk2 = np.ascontiguousarray(np.concatenate([Wk.T, Wk.T], axis=1))
    wvb = np.ascontiguousarray(Wv.T.astype(ml_dtypes.bfloat16))
    mkb = np.triu(np.ones((128, 128), dtype=np.float32)).astype(
        ml_dtypes.bfloat16)
    onesb = np.ones((128, NSB), dtype=np.float32).astype(ml_dtypes.bfloat16)

    nc = _get_nc()
    in_maps = []
    for c in range(NCORES):
        in_maps.append({
            "xt": np.ascontiguousarray(xT[BPC * c: BPC * (c + 1)]),
            "xbf": np.ascontiguousarray(xbf[BPC * c: BPC * (c + 1)]),
            "wqt2": wq2, "wkt2": wk2, "wvb": wvb,
            "mkb": mkb, "onesb": onesb,
        })
    import os
    kw = {}
    if os.environ.get("BASS_TRACE"):
        kw = dict(trace=True, stitch_traces=False)
    res = run_bass_kernel_spmd(nc, in_maps, core_ids=list(range(NCORES)), **kw)
    _cache["last_result"] = res
    out = np.empty((B, T, D), dtype=np.float32)
    for c in range(NCORES):
        # device layout [BPC, NTT, p(128), tch(4), h] -> [BPC, T, D]
        o = res.results[c]["out"].transpose(0, 1, 3, 2, 4).reshape(BPC, T, D)
        out[BPC * c: BPC * (c + 1)] = o
    return out
